# revision 20
# baseline (speedup 1.0000x reference)
"""nn_CustomLSTMModel kernel: 2-layer LSTM (H=1024) over (B=64, T=512) tokens.

Strategy: tensor-parallel over the gate/hidden dimension across 8 NeuronCores.
Each core j owns hidden slice [j*128, (j+1)*128) of both layers: it holds the
weight rows for its slice (transposed, bf16), computes its slice of the gates
for the FULL batch (64), updates its local cell-state slice, and exchanges its
h-slice with the other cores via one AllGather per superstep (both layers'
chunks ride one collective).  The layers run as a lag-2 wavefront: superstep s
computes h0(s) and h1(s-2), so layer 1's input-side matmuls (on h0(s-2)) and
layer 0's input-side matmuls (on the pre-gathered token embeddings) are not
gated on the in-flight AllGather — they fill the PE during the collective and
keep the HAM clock warm.  Embeddings are gathered on device (indirect DMA)
and PE-transposed into an SBUF ring ahead of consumption.  The final vocab
projection is sharded over cores (4000 rows each); the host concatenates.

Weights stream through the PE as the moving operand in bf16 (1 cycle/row);
h stays transposed [H, B] in SBUF so it can be the stationary operand.
Cell state c stays fp32 and core-local (never communicated).
"""

import os
import sys
import numpy as np

for _p in ("/opt/trn_rl_repo", "/opt/pypackages"):
    if _p not in sys.path:
        sys.path.insert(0, _p)

import ml_dtypes

BF16 = ml_dtypes.bfloat16

VOCAB, EMBED, HIDDEN, BATCH, SEQ = 32000, 512, 1024, 64, 512
NCORES = 8
HS = HIDDEN // NCORES           # 128: hidden slice per core
GS = 4 * HS                     # 512: gate rows per core (f,i,o,c x 128)
VS = VOCAB // NCORES            # 4000: vocab slice per core
KH = HIDDEN // 128              # 8 contraction chunks over H
KE = EMBED // 128               # 4 contraction chunks over E
NSTEPS = int(os.environ.get("LSTM_STEPS", str(SEQ)))
LOOKAHEAD = 6                   # prologue tiles (2 steps each) emitted ahead

TRACE = False          # set by test.py; when True, run traced, set LAST_EXEC_NS
LAST_EXEC_NS = None

_BUILT = None          # cached {key: nc}


# ---------------------------------------------------------------- host prep

def _prep_shards(Wf0, Wi0, Wo0, Wc0, bf0, bi0, bo0, bc0,
                 Wf1, Wi1, Wo1, Wc1, bf1, bi1, bo1, bc1, Wy, by):
    """Build per-core weight shards in the on-chip layouts."""
    gates0 = [np.asarray(w, np.float32) for w in (Wf0, Wi0, Wo0, Wc0)]
    gates1 = [np.asarray(w, np.float32) for w in (Wf1, Wi1, Wo1, Wc1)]
    b0 = [np.asarray(b, np.float32) for b in (bf0, bi0, bo0, bc0)]
    b1 = [np.asarray(b, np.float32) for b in (bf1, bi1, bo1, bc1)]
    Wy = np.asarray(Wy, np.float32)
    by = np.asarray(by, np.float32)

    def chunked(W):  # [K, N] -> [128, (K//128)*N], chunk k at cols [k*N,(k+1)*N)
        K, N = W.shape
        return np.ascontiguousarray(
            W.reshape(K // 128, 128, N).transpose(1, 0, 2).reshape(128, -1)
        )

    shards = []
    for j in range(NCORES):
        r = slice(j * HS, (j + 1) * HS)
        Wh0 = np.concatenate([g[r, :HIDDEN].T for g in gates0], axis=1)  # (1024,512)
        Wx0 = np.concatenate([g[r, HIDDEN:].T for g in gates0], axis=1)  # (512,512)
        Wh1 = np.concatenate([g[r, :HIDDEN].T for g in gates1], axis=1)  # (1024,512)
        Wx1 = np.concatenate([g[r, HIDDEN:].T for g in gates1], axis=1)  # (1024,512)
        b0row = np.concatenate([b[r] for b in b0])[None, :]
        b1row = np.concatenate([b[r] for b in b1])[None, :]

        v = slice(j * VS, (j + 1) * VS)
        WYj = np.ascontiguousarray(
            Wy[v, :].T.reshape(KH, 128, VS).transpose(1, 0, 2).reshape(128, -1)
        )
        byrow = by[v][None, :]

        shards.append(dict(
            WH0=chunked(Wh0).astype(BF16), WX0=chunked(Wx0).astype(BF16),
            WH1=chunked(Wh1).astype(BF16), WX1=chunked(Wx1).astype(BF16),
            B0=b0row.astype(BF16), B1=b1row.astype(BF16),
            WY=WYj.astype(BF16), BY=byrow.astype(BF16),
        ))
    has_b0 = any(np.any(b != 0) for b in b0)
    has_b1 = any(np.any(b != 0) for b in b1)
    has_by = bool(np.any(by != 0))
    return shards, (has_b0, has_b1, has_by)


# ---------------------------------------------------------------- builder

def _build(nsteps, has_b0, has_b1, has_by):
    import concourse.bacc as bacc
    import concourse.bass as bass
    import concourse.mybir as mybir
    import concourse.tile as tile

    dt = mybir.dt
    AF = mybir.ActivationFunctionType
    NTOK = nsteps * BATCH
    NTILES = (NTOK + 127) // 128          # 128-token prologue tiles (2 steps)

    nc = bacc.Bacc("TRN2", target_bir_lowering=False, debug=False,
                   enable_asserts=True, num_devices=NCORES)

    offs = nc.dram_tensor("offs", [NTILES * 128], dt.int32, kind="ExternalInput")
    emb_bf = nc.dram_tensor("emb_bf", [VOCAB, EMBED], dt.bfloat16,
                            kind="ExternalInput")
    WH0 = nc.dram_tensor("WH0", [128, KH * GS], dt.bfloat16, kind="ExternalInput")
    WX0 = nc.dram_tensor("WX0", [128, KE * GS], dt.bfloat16, kind="ExternalInput")
    WH1 = nc.dram_tensor("WH1", [128, KH * GS], dt.bfloat16, kind="ExternalInput")
    WX1 = nc.dram_tensor("WX1", [128, KH * GS], dt.bfloat16, kind="ExternalInput")
    B0 = nc.dram_tensor("B0", [1, GS], dt.bfloat16, kind="ExternalInput")
    B1 = nc.dram_tensor("B1", [1, GS], dt.bfloat16, kind="ExternalInput")
    WY = nc.dram_tensor("WY", [128, KH * VS], dt.bfloat16, kind="ExternalInput")
    BY = nc.dram_tensor("BY", [1, VS], dt.bfloat16, kind="ExternalInput")
    ONES = nc.dram_tensor("ONES", [1, 128], dt.bfloat16, kind="ExternalInput")
    IDENT = nc.dram_tensor("IDENT", [128, 128], dt.bfloat16, kind="ExternalInput")
    y_out = nc.dram_tensor("y", [BATCH, VS], dt.float32, kind="ExternalOutput")

    with tile.TileContext(nc) as tc:
        with (
            tc.tile_pool(name="wpool", bufs=1) as wpool,
            tc.tile_pool(name="state", bufs=1) as state,
            tc.tile_pool(name="work", bufs=3) as work,
            tc.tile_pool(name="xring", bufs=LOOKAHEAD + 3) as xring,
            tc.tile_pool(name="hbuf", bufs=4) as hbuf,
            tc.tile_pool(name="psum", bufs=2, space="PSUM") as psum,
            tc.tile_pool(name="tpsum", bufs=4, space="PSUM") as tpsum,
            tc.tile_pool(name="agd", bufs=2, space="DRAM") as agd,
        ):
            # ---- persistent weights/constants ----
            def load(tname, src, shape, dtype=dt.bfloat16):
                t = wpool.tile(shape, dtype, tag=tname)
                nc.sync.dma_start(t[:], src.ap())
                return t

            wh0 = load("wh0", WH0, [128, KH * GS])
            wx0 = load("wx0", WX0, [128, KE * GS])
            wh1 = load("wh1", WH1, [128, KH * GS])
            wx1 = load("wx1", WX1, [128, KH * GS])
            b0r = load("b0r", B0, [1, GS])
            b1r = load("b1r", B1, [1, GS])
            wy = load("wy", WY, [128, KH * VS])
            byr = load("byr", BY, [1, VS])
            ones = load("ones", ONES, [1, 128])
            ident = load("ident", IDENT, [128, 128])
            offs_t = wpool.tile([128, NTILES], dt.int32, tag="offs")
            nc.sync.dma_start(
                offs_t[:], offs.ap().rearrange("(t p) -> p t", p=128))

            # ---- persistent state ----
            c0 = state.tile([BATCH, HS], dt.float32, tag="c0")
            c1 = state.tile([BATCH, HS], dt.float32, tag="c1")
            zro = state.tile([128, BATCH], dt.bfloat16, tag="zro")
            nc.vector.memset(c0[:], 0.0)
            nc.vector.memset(c1[:], 0.0)
            nc.vector.memset(zro[:], 0.0)

            h0T_init = hbuf.tile([128, KH * BATCH], dt.bfloat16, tag="h0T")
            h1T_init = hbuf.tile([128, KH * BATCH], dt.bfloat16, tag="h1T")
            nc.vector.memset(h0T_init[:], 0.0)
            nc.vector.memset(h1T_init[:], 0.0)

            x_tiles = [None] * NTILES

            # ---- prologue tile: gather 128 tokens, transpose to xsT ring ----
            def emit_x_tile(tau):
                ntok = min(128, NTOK - tau * 128)
                xg = work.tile([128, EMBED], dt.bfloat16, tag="xg")
                nc.gpsimd.indirect_dma_start(
                    out=xg[:ntok, :], out_offset=None,
                    in_=emb_bf.ap(),
                    in_offset=bass.IndirectOffsetOnAxis(
                        ap=offs_t[:ntok, tau:tau + 1], axis=0),
                )
                xT = xring.tile([128, KE * 128], dt.bfloat16, tag="xT")
                for ch in range(KE):
                    tp = tpsum.tile([128, 128], dt.bfloat16, tag="tp")
                    nc.tensor.transpose(
                        out=tp[:, :ntok], in_=xg[:ntok, ch * 128:(ch + 1) * 128],
                        identity=ident[:ntok, :ntok])
                    nc.vector.tensor_copy(
                        xT[:, ch * 128:ch * 128 + ntok], tp[:, :ntok])
                x_tiles[tau] = xT

            for tau in range(min(LOOKAHEAD, NTILES)):
                emit_x_tile(tau)

            # ---- EW: gates PSUM [64, GS] -> h chunk bf16 + c update ----
            def lstm_ew(ps, c, tag):
                sio = work.tile([BATCH, 3 * HS], dt.float32, tag=f"sio{tag}")
                th = work.tile([BATCH, HS], dt.float32, tag=f"th{tag}")
                nc.scalar.activation(sio[:], ps[:, 0:3 * HS], AF.Sigmoid)
                nc.scalar.activation(th[:], ps[:, 3 * HS:4 * HS], AF.Tanh)
                t1 = work.tile([BATCH, HS], dt.float32, tag=f"t1{tag}")
                nc.vector.tensor_mul(t1[:], sio[:, 0:HS], c[:])          # f*c
                t2 = work.tile([BATCH, HS], dt.float32, tag=f"t2{tag}")
                nc.vector.tensor_mul(t2[:], sio[:, HS:2 * HS], th[:])    # i*ch
                nc.vector.tensor_add(c[:], t1[:], t2[:])
                tc_ = work.tile([BATCH, HS], dt.float32, tag=f"tc{tag}")
                nc.scalar.activation(tc_[:], c[:], AF.Tanh)
                hj = work.tile([BATCH, HS], dt.bfloat16, tag=f"hj{tag}")
                nc.vector.tensor_mul(hj[:], sio[:, 2 * HS:3 * HS], tc_[:])
                tp = tpsum.tile([128, BATCH], dt.bfloat16, tag="tp")
                nc.tensor.transpose(out=tp[:], in_=hj[:],
                                    identity=ident[:BATCH, :BATCH])
                hT = work.tile([128, BATCH], dt.bfloat16, tag=f"hT{tag}")
                nc.vector.tensor_copy(hT[:], tp[:])
                return hT

            # ---- recurrence: superstep s computes h0(s) and h1(s-2) ----
            h0T_p1 = h0T_init   # h0T from AG(s-1)
            h0T_p2 = h0T_init   # h0T from AG(s-2)
            h1T_p1 = h1T_init   # h1T from AG(s-1) = h1(s-3)
            nsup = nsteps + 2
            for s in range(1, nsup + 1):
                agin = agd.tile([2 * 128, BATCH], dt.bfloat16, tag="agin")
                agout = agd.tile([NCORES * 2 * 128, BATCH], dt.bfloat16,
                                 tag="agout")

                do_l0 = s <= nsteps
                do_l1 = s >= 3

                # ---------- ungated work (runs during AG(s-1) flight) ----------
                if do_l1:
                    # layer 1, step s-2: x-part = hs0(s-2) @ Wx1
                    ps1 = psum.tile([BATCH, GS], dt.float32, tag="g1ps")
                    for k in range(KH):
                        nc.tensor.matmul(
                            ps1[:], lhsT=h0T_p2[:, k * BATCH:(k + 1) * BATCH],
                            rhs=wx1[:, k * GS:(k + 1) * GS],
                            start=(k == 0), stop=False)
                    if has_b1:
                        nc.tensor.matmul(ps1[:], lhsT=ones[:, :BATCH],
                                         rhs=b1r[:], start=False, stop=False)
                if do_l0:
                    # layer 0, step s: x-part = xs(s) @ Wx0 (+ b0)
                    ps0 = psum.tile([BATCH, GS], dt.float32, tag="g0ps")
                    tau, off = (s - 1) // 2, ((s - 1) % 2) * BATCH
                    xT = x_tiles[tau]
                    for ch in range(KE):
                        nc.tensor.matmul(
                            ps0[:], lhsT=xT[:, ch * 128 + off:ch * 128 + off + BATCH],
                            rhs=wx0[:, ch * GS:(ch + 1) * GS],
                            start=(ch == 0), stop=False)
                    if has_b0:
                        nc.tensor.matmul(ps0[:], lhsT=ones[:, :BATCH],
                                         rhs=b0r[:], start=False, stop=False)

                # ---------- gated on readback of AG(s-1) ----------
                # L1 first: its EW chain finishes last and gates the AG
                # trigger, so give it the head start on PE/ACT/DVE.
                if do_l1:
                    # layer 1 h-part: h1(s-3) @ Wh1  (h1T_p1 from AG(s-1))
                    for k in range(KH):
                        nc.tensor.matmul(
                            ps1[:], lhsT=h1T_p1[:, k * BATCH:(k + 1) * BATCH],
                            rhs=wh1[:, k * GS:(k + 1) * GS],
                            start=False, stop=(k == KH - 1))
                    hT1 = lstm_ew(ps1, c1, "1")
                    nc.sync.dma_start(agin[128:256, :], hT1[:])
                else:
                    nc.sync.dma_start(agin[128:256, :], zro[:])

                if do_l0:
                    for k in range(KH):
                        nc.tensor.matmul(
                            ps0[:], lhsT=h0T_p1[:, k * BATCH:(k + 1) * BATCH],
                            rhs=wh0[:, k * GS:(k + 1) * GS],
                            start=False, stop=(k == KH - 1))
                    hT0 = lstm_ew(ps0, c0, "0")
                    nc.sync.dma_start(agin[0:128, :], hT0[:])
                else:
                    nc.sync.dma_start(agin[0:128, :], zro[:])

                nc.gpsimd.collective_compute(
                    "AllGather", mybir.AluOpType.bypass,
                    replica_groups=[list(range(NCORES))],
                    ins=[agin.opt()], outs=[agout.opt()])

                h0T_new = hbuf.tile([128, KH * BATCH], dt.bfloat16, tag="h0T")
                h1T_new = hbuf.tile([128, KH * BATCH], dt.bfloat16, tag="h1T")
                v = agout[:].rearrange("(r two p) b -> two p r b",
                                       r=NCORES, two=2, p=128)
                d0 = h0T_new[:].rearrange("p (r b) -> p r b", r=NCORES)
                d1 = h1T_new[:].rearrange("p (r b) -> p r b", r=NCORES)
                nc.sync.dma_start(d1, v[1])      # h1 first: gates next L1h
                nc.scalar.dma_start(d0, v[0])    # parallel queue
                h0T_p2, h0T_p1 = h0T_p1, h0T_new
                h1T_p1 = h1T_new

                # gather lands behind the collective on the gpsimd queue, so
                # it runs during the AG flight instead of delaying the trigger
                tau_need = LOOKAHEAD + (s - 1) // 2
                if tau_need < NTILES and x_tiles[tau_need] is None:
                    emit_x_tile(tau_need)

            # ---- final projection: y_j = h1(T) @ Wy_j.T + by_j ----
            NCH = (VS + 499) // 500
            ysb = state.tile([BATCH, VS], dt.float32, tag="ysb")
            for nchunk in range(NCH):
                n0 = nchunk * 500
                n1 = min(n0 + 500, VS)
                w = n1 - n0
                yps = psum.tile([BATCH, 500], dt.float32, tag="g0ps")
                for k in range(KH):
                    nc.tensor.matmul(
                        yps[:, :w], lhsT=h1T_p1[:, k * BATCH:(k + 1) * BATCH],
                        rhs=wy[:, k * VS + n0:k * VS + n1],
                        start=(k == 0), stop=(not has_by and k == KH - 1))
                if has_by:
                    nc.tensor.matmul(yps[:, :w], lhsT=ones[:, :BATCH],
                                     rhs=byr[:, n0:n1], start=False, stop=True)
                nc.vector.tensor_copy(ysb[:, n0:n1], yps[:, :w])
            nc.sync.dma_start(y_out.ap(), ysb[:])

    nc.compile()
    return nc


def _get_built(flags):
    global _BUILT
    if _BUILT is None:
        _BUILT = _build(NSTEPS, *flags)
    return _BUILT


# ---------------------------------------------------------------- runner

def _install_trace_hook():
    import types
    try:
        from trn_agent_boot.trn_boot import _ntff_profile_via_ctypes
        import antenv  # noqa: F401
        hook = _ntff_profile_via_ctypes('/opt/axon/libaxon_pjrt.so')
        mod = types.ModuleType('antenv.axon_hooks')
        mod.get_axon_ntff_profile_hook = lambda: hook
        sys.modules['antenv.axon_hooks'] = mod
        return True
    except Exception:
        return False


def kernel(texts, emb, Wf0, bf0, Wi0, bi0, Wo0, bo0, Wc0, bc0,
           Wf1, bf1, Wi1, bi1, Wo1, bo1, Wc1, bc1, Wy, by):
    global LAST_EXEC_NS
    from concourse import bass_utils

    texts = np.asarray(texts)
    emb = np.asarray(emb, np.float32)
    shards, flags = _prep_shards(Wf0, Wi0, Wo0, Wc0, bf0, bi0, bo0, bc0,
                                 Wf1, Wi1, Wo1, Wc1, bf1, bi1, bo1, bc1,
                                 Wy, by)

    nsteps = NSTEPS
    ntok = nsteps * BATCH
    ntiles = (ntok + 127) // 128
    offs = np.zeros(ntiles * 128, np.int32)
    offs[:ntok] = texts.T[:nsteps, :].reshape(-1).astype(np.int32)
    emb_bf = emb.astype(BF16)
    ones = np.ones((1, 128), BF16)
    ident = np.eye(128, dtype=BF16)

    in_maps = []
    for j in range(NCORES):
        sh = shards[j]
        in_maps.append({
            "offs": offs, "emb_bf": emb_bf,
            "WH0": sh["WH0"], "WX0": sh["WX0"],
            "WH1": sh["WH1"], "WX1": sh["WX1"],
            "B0": sh["B0"], "B1": sh["B1"],
            "WY": sh["WY"], "BY": sh["BY"],
            "ONES": ones, "IDENT": ident,
        })

    nc = _get_built(flags)
    kwargs = {}
    if TRACE:
        if _install_trace_hook():
            import tempfile
            kwargs = dict(trace=True,
                          tmpdir=tempfile.mkdtemp(prefix="lstm_trace_"))
    res = bass_utils.run_bass_kernel_spmd(
        nc, in_maps, core_ids=list(range(NCORES)), **kwargs)
    LAST_EXEC_NS = res.exec_time_ns
    y = np.concatenate([res.results[j]["y"] for j in range(NCORES)], axis=1)
    return y.astype(np.float32)


if __name__ == "__main__":
    pass


# revision 23
# speedup vs baseline: 1.0006x; 1.0006x over previous
"""nn_CustomLSTMModel kernel: 2-layer LSTM (H=1024) over (B=64, T=512) tokens.

Strategy: tensor-parallel over the gate/hidden dimension across 8 NeuronCores.
Each core j owns hidden slice [j*128, (j+1)*128) of both layers: it holds the
weight rows for its slice (transposed, bf16), computes its slice of the gates
for the FULL batch (64), updates its local cell-state slice, and exchanges its
h-slice with the other cores via one AllGather per superstep (both layers'
chunks ride one collective).  The layers run as a lag-2 wavefront: superstep s
computes h0(s) and h1(s-2), so layer 1's input-side matmuls (on h0(s-2)) and
layer 0's input-side matmuls (on the pre-gathered token embeddings) are not
gated on the in-flight AllGather — they fill the PE during the collective and
keep the HAM clock warm.  Embeddings are gathered on device (indirect DMA)
and PE-transposed into an SBUF ring ahead of consumption.  The final vocab
projection is sharded over cores (4000 rows each); the host concatenates.

Weights stream through the PE as the moving operand in bf16 (1 cycle/row);
h stays transposed [H, B] in SBUF so it can be the stationary operand.
Cell state c stays fp32 and core-local (never communicated).
"""

import os
import sys
import numpy as np

for _p in ("/opt/trn_rl_repo", "/opt/pypackages"):
    if _p not in sys.path:
        sys.path.insert(0, _p)

import ml_dtypes

BF16 = ml_dtypes.bfloat16

VOCAB, EMBED, HIDDEN, BATCH, SEQ = 32000, 512, 1024, 64, 512
NCORES = 8
HS = HIDDEN // NCORES           # 128: hidden slice per core
GS = 4 * HS                     # 512: gate rows per core (f,i,o,c x 128)
VS = VOCAB // NCORES            # 4000: vocab slice per core
KH = HIDDEN // 128              # 8 contraction chunks over H
KE = EMBED // 128               # 4 contraction chunks over E
NSTEPS = int(os.environ.get("LSTM_STEPS", str(SEQ)))
LOOKAHEAD = 6                   # prologue tiles (2 steps each) emitted ahead

TRACE = False          # set by test.py; when True, run traced, set LAST_EXEC_NS
LAST_EXEC_NS = None

_BUILT = None          # cached {key: nc}


# ---------------------------------------------------------------- host prep

def _prep_shards(Wf0, Wi0, Wo0, Wc0, bf0, bi0, bo0, bc0,
                 Wf1, Wi1, Wo1, Wc1, bf1, bi1, bo1, bc1, Wy, by):
    """Build per-core weight shards in the on-chip layouts."""
    gates0 = [np.asarray(w, np.float32) for w in (Wf0, Wi0, Wo0, Wc0)]
    gates1 = [np.asarray(w, np.float32) for w in (Wf1, Wi1, Wo1, Wc1)]
    b0 = [np.asarray(b, np.float32) for b in (bf0, bi0, bo0, bc0)]
    b1 = [np.asarray(b, np.float32) for b in (bf1, bi1, bo1, bc1)]
    Wy = np.asarray(Wy, np.float32)
    by = np.asarray(by, np.float32)

    def chunked(W):  # [K, N] -> [128, (K//128)*N], chunk k at cols [k*N,(k+1)*N)
        K, N = W.shape
        return np.ascontiguousarray(
            W.reshape(K // 128, 128, N).transpose(1, 0, 2).reshape(128, -1)
        )

    shards = []
    for j in range(NCORES):
        r = slice(j * HS, (j + 1) * HS)
        Wh0 = np.concatenate([g[r, :HIDDEN].T for g in gates0], axis=1)  # (1024,512)
        Wx0 = np.concatenate([g[r, HIDDEN:].T for g in gates0], axis=1)  # (512,512)
        Wh1 = np.concatenate([g[r, :HIDDEN].T for g in gates1], axis=1)  # (1024,512)
        Wx1 = np.concatenate([g[r, HIDDEN:].T for g in gates1], axis=1)  # (1024,512)
        b0row = np.concatenate([b[r] for b in b0])[None, :]
        b1row = np.concatenate([b[r] for b in b1])[None, :]

        v = slice(j * VS, (j + 1) * VS)
        WYj = np.ascontiguousarray(
            Wy[v, :].T.reshape(KH, 128, VS).transpose(1, 0, 2).reshape(128, -1)
        )
        byrow = by[v][None, :]

        shards.append(dict(
            WH0=chunked(Wh0).astype(BF16), WX0=chunked(Wx0).astype(BF16),
            WH1=chunked(Wh1).astype(BF16), WX1=chunked(Wx1).astype(BF16),
            B0=b0row.astype(BF16), B1=b1row.astype(BF16),
            WY=WYj.astype(BF16), BY=byrow.astype(BF16),
        ))
    has_b0 = any(np.any(b != 0) for b in b0)
    has_b1 = any(np.any(b != 0) for b in b1)
    has_by = bool(np.any(by != 0))
    return shards, (has_b0, has_b1, has_by)


# ---------------------------------------------------------------- builder

def _build(nsteps, has_b0, has_b1, has_by):
    import concourse.bacc as bacc
    import concourse.bass as bass
    import concourse.mybir as mybir
    import concourse.tile as tile

    dt = mybir.dt
    AF = mybir.ActivationFunctionType
    NTOK = nsteps * BATCH
    NTILES = (NTOK + 127) // 128          # 128-token prologue tiles (2 steps)

    nc = bacc.Bacc("TRN2", target_bir_lowering=False, debug=False,
                   enable_asserts=True, num_devices=NCORES)

    offs = nc.dram_tensor("offs", [NTILES * 128], dt.int32, kind="ExternalInput")
    emb_bf = nc.dram_tensor("emb_bf", [VOCAB, EMBED], dt.bfloat16,
                            kind="ExternalInput")
    WH0 = nc.dram_tensor("WH0", [128, KH * GS], dt.bfloat16, kind="ExternalInput")
    WX0 = nc.dram_tensor("WX0", [128, KE * GS], dt.bfloat16, kind="ExternalInput")
    WH1 = nc.dram_tensor("WH1", [128, KH * GS], dt.bfloat16, kind="ExternalInput")
    WX1 = nc.dram_tensor("WX1", [128, KH * GS], dt.bfloat16, kind="ExternalInput")
    B0 = nc.dram_tensor("B0", [1, GS], dt.bfloat16, kind="ExternalInput")
    B1 = nc.dram_tensor("B1", [1, GS], dt.bfloat16, kind="ExternalInput")
    WY = nc.dram_tensor("WY", [128, KH * VS], dt.bfloat16, kind="ExternalInput")
    BY = nc.dram_tensor("BY", [1, VS], dt.bfloat16, kind="ExternalInput")
    ONES = nc.dram_tensor("ONES", [1, 128], dt.bfloat16, kind="ExternalInput")
    IDENT = nc.dram_tensor("IDENT", [128, 128], dt.bfloat16, kind="ExternalInput")
    y_out = nc.dram_tensor("y", [BATCH, VS], dt.float32, kind="ExternalOutput")

    with tile.TileContext(nc) as tc:
        with (
            tc.tile_pool(name="wpool", bufs=1) as wpool,
            tc.tile_pool(name="state", bufs=1) as state,
            tc.tile_pool(name="work", bufs=3) as work,
            tc.tile_pool(name="xring", bufs=LOOKAHEAD + 3) as xring,
            tc.tile_pool(name="hbuf", bufs=4) as hbuf,
            tc.tile_pool(name="psum", bufs=2, space="PSUM") as psum,
            tc.tile_pool(name="tpsum", bufs=4, space="PSUM") as tpsum,
            tc.tile_pool(name="agd", bufs=2, space="DRAM") as agd,
        ):
            # ---- persistent weights/constants ----
            def load(tname, src, shape, dtype=dt.bfloat16):
                t = wpool.tile(shape, dtype, tag=tname)
                nc.sync.dma_start(t[:], src.ap())
                return t

            wh0 = load("wh0", WH0, [128, KH * GS])
            wx0 = load("wx0", WX0, [128, KE * GS])
            wh1 = load("wh1", WH1, [128, KH * GS])
            wx1 = load("wx1", WX1, [128, KH * GS])
            b0r = load("b0r", B0, [1, GS])
            b1r = load("b1r", B1, [1, GS])
            wy = load("wy", WY, [128, KH * VS])
            byr = load("byr", BY, [1, VS])
            ones = load("ones", ONES, [1, 128])
            ident = load("ident", IDENT, [128, 128])
            offs_t = wpool.tile([128, NTILES], dt.int32, tag="offs")
            nc.sync.dma_start(
                offs_t[:], offs.ap().rearrange("(t p) -> p t", p=128))

            # ---- persistent state ----
            c0 = state.tile([BATCH, HS], dt.float32, tag="c0")
            c1 = state.tile([BATCH, HS], dt.float32, tag="c1")
            zro = state.tile([128, BATCH], dt.bfloat16, tag="zro")
            nc.vector.memset(c0[:], 0.0)
            nc.vector.memset(c1[:], 0.0)
            nc.vector.memset(zro[:], 0.0)

            h0T_init = hbuf.tile([128, KH * BATCH], dt.bfloat16, tag="h0T")
            h1T_init = hbuf.tile([128, KH * BATCH], dt.bfloat16, tag="h1T")
            nc.vector.memset(h0T_init[:], 0.0)
            nc.vector.memset(h1T_init[:], 0.0)

            x_tiles = [None] * NTILES

            # ---- prologue tile: gather 128 tokens, transpose to xsT ring ----
            def emit_x_tile(tau):
                ntok = min(128, NTOK - tau * 128)
                xg = work.tile([128, EMBED], dt.bfloat16, tag="xg")
                nc.gpsimd.indirect_dma_start(
                    out=xg[:ntok, :], out_offset=None,
                    in_=emb_bf.ap(),
                    in_offset=bass.IndirectOffsetOnAxis(
                        ap=offs_t[:ntok, tau:tau + 1], axis=0),
                )
                xT = xring.tile([128, KE * 128], dt.bfloat16, tag="xT")
                for ch in range(KE):
                    tp = tpsum.tile([128, 128], dt.bfloat16, tag="tp")
                    nc.tensor.transpose(
                        out=tp[:, :ntok], in_=xg[:ntok, ch * 128:(ch + 1) * 128],
                        identity=ident[:ntok, :ntok])
                    nc.vector.tensor_copy(
                        xT[:, ch * 128:ch * 128 + ntok], tp[:, :ntok])
                x_tiles[tau] = xT

            for tau in range(min(LOOKAHEAD, NTILES)):
                emit_x_tile(tau)

            # ---- EW: gates PSUM [64, GS] -> h chunk bf16 + c update ----
            def lstm_ew(ps, c, tag):
                sio = work.tile([BATCH, 3 * HS], dt.float32, tag=f"sio{tag}")
                th = work.tile([BATCH, HS], dt.float32, tag=f"th{tag}")
                nc.scalar.activation(sio[:], ps[:, 0:3 * HS], AF.Sigmoid)
                nc.scalar.activation(th[:], ps[:, 3 * HS:4 * HS], AF.Tanh)
                t1 = work.tile([BATCH, HS], dt.float32, tag=f"t1{tag}")
                nc.vector.tensor_mul(t1[:], sio[:, 0:HS], c[:])          # f*c
                t2 = work.tile([BATCH, HS], dt.float32, tag=f"t2{tag}")
                nc.vector.tensor_mul(t2[:], sio[:, HS:2 * HS], th[:])    # i*ch
                nc.vector.tensor_add(c[:], t1[:], t2[:])
                tc_ = work.tile([BATCH, HS], dt.float32, tag=f"tc{tag}")
                nc.scalar.activation(tc_[:], c[:], AF.Tanh)
                hj = work.tile([BATCH, HS], dt.bfloat16, tag=f"hj{tag}")
                nc.vector.tensor_mul(hj[:], sio[:, 2 * HS:3 * HS], tc_[:])
                tp = tpsum.tile([128, BATCH], dt.bfloat16, tag="tp")
                nc.tensor.transpose(out=tp[:], in_=hj[:],
                                    identity=ident[:BATCH, :BATCH])
                hT = work.tile([128, BATCH], dt.bfloat16, tag=f"hT{tag}")
                nc.vector.tensor_copy(hT[:], tp[:])
                return hT

            # ---- recurrence: superstep s computes h0(s) and h1(s-2) ----
            h0T_p1 = h0T_init   # h0T from AG(s-1)
            h0T_p2 = h0T_init   # h0T from AG(s-2)
            h1T_p1 = h1T_init   # h1T from AG(s-1) = h1(s-3)
            nsup = nsteps + 2
            for s in range(1, nsup + 1):
                do_l0 = s <= nsteps
                do_l1 = s >= 3

                # ---------- ungated work (runs during AG(s-1) flight) ----------
                if do_l1:
                    # layer 1, step s-2: x-part = hs0(s-2) @ Wx1
                    ps1 = psum.tile([BATCH, GS], dt.float32, tag="g1ps")
                    for k in range(KH):
                        nc.tensor.matmul(
                            ps1[:], lhsT=h0T_p2[:, k * BATCH:(k + 1) * BATCH],
                            rhs=wx1[:, k * GS:(k + 1) * GS],
                            start=(k == 0), stop=False)
                    if has_b1:
                        nc.tensor.matmul(ps1[:], lhsT=ones[:, :BATCH],
                                         rhs=b1r[:], start=False, stop=False)
                if do_l0:
                    # layer 0, step s: x-part = xs(s) @ Wx0 (+ b0)
                    ps0 = psum.tile([BATCH, GS], dt.float32, tag="g0ps")
                    tau, off = (s - 1) // 2, ((s - 1) % 2) * BATCH
                    xT = x_tiles[tau]
                    for ch in range(KE):
                        nc.tensor.matmul(
                            ps0[:], lhsT=xT[:, ch * 128 + off:ch * 128 + off + BATCH],
                            rhs=wx0[:, ch * GS:(ch + 1) * GS],
                            start=(ch == 0), stop=False)
                    if has_b0:
                        nc.tensor.matmul(ps0[:], lhsT=ones[:, :BATCH],
                                         rhs=b0r[:], start=False, stop=False)

                # ---------- gated on readbacks of AG0/AG1(s-1) ----------
                # L1 first: its AG1 triggers as soon as EW1 finishes; AG1's
                # result is only needed next superstep, so its latency hides.
                if do_l1:
                    # layer 1 h-part: h1(s-3) @ Wh1  (h1T_p1 from AG1(s-1))
                    for k in range(KH):
                        nc.tensor.matmul(
                            ps1[:], lhsT=h1T_p1[:, k * BATCH:(k + 1) * BATCH],
                            rhs=wh1[:, k * GS:(k + 1) * GS],
                            start=False, stop=(k == KH - 1))
                    hT1 = lstm_ew(ps1, c1, "1")
                    agin1 = agd.tile([128, BATCH], dt.bfloat16, tag="agin1")
                    agout1 = agd.tile([NCORES * 128, BATCH], dt.bfloat16,
                                      tag="agout1")
                    nc.sync.dma_start(agin1[:], hT1[:])
                    nc.gpsimd.collective_compute(
                        "AllGather", mybir.AluOpType.bypass,
                        replica_groups=[list(range(NCORES))],
                        ins=[agin1.opt()], outs=[agout1.opt()])
                    h1T_new = hbuf.tile([128, KH * BATCH], dt.bfloat16,
                                        tag="h1T")
                    nc.sync.dma_start(
                        h1T_new[:].rearrange("p (r b) -> p r b", r=NCORES),
                        agout1[:].rearrange("(r p) b -> p r b",
                                            r=NCORES, p=128))
                    h1T_p1 = h1T_new

                if do_l0:
                    for k in range(KH):
                        nc.tensor.matmul(
                            ps0[:], lhsT=h0T_p1[:, k * BATCH:(k + 1) * BATCH],
                            rhs=wh0[:, k * GS:(k + 1) * GS],
                            start=False, stop=(k == KH - 1))
                    hT0 = lstm_ew(ps0, c0, "0")
                    agin0 = agd.tile([128, BATCH], dt.bfloat16, tag="agin0")
                    agout0 = agd.tile([NCORES * 128, BATCH], dt.bfloat16,
                                      tag="agout0")
                    nc.sync.dma_start(agin0[:], hT0[:])
                    nc.gpsimd.collective_compute(
                        "AllGather", mybir.AluOpType.bypass,
                        replica_groups=[list(range(NCORES))],
                        ins=[agin0.opt()], outs=[agout0.opt()])
                    h0T_new = hbuf.tile([128, KH * BATCH], dt.bfloat16,
                                        tag="h0T")
                    nc.scalar.dma_start(
                        h0T_new[:].rearrange("p (r b) -> p r b", r=NCORES),
                        agout0[:].rearrange("(r p) b -> p r b",
                                            r=NCORES, p=128))
                    h0T_p2, h0T_p1 = h0T_p1, h0T_new
                else:
                    h0T_p2 = h0T_p1

                # gather lands behind the collective on the gpsimd queue, so
                # it runs during the AG flight instead of delaying the trigger
                tau_need = LOOKAHEAD + (s - 1) // 2
                if tau_need < NTILES and x_tiles[tau_need] is None:
                    emit_x_tile(tau_need)

            # ---- final projection: y_j = h1(T) @ Wy_j.T + by_j ----
            NCH = (VS + 499) // 500
            ysb = state.tile([BATCH, VS], dt.float32, tag="ysb")
            for nchunk in range(NCH):
                n0 = nchunk * 500
                n1 = min(n0 + 500, VS)
                w = n1 - n0
                yps = psum.tile([BATCH, 500], dt.float32, tag="g0ps")
                for k in range(KH):
                    nc.tensor.matmul(
                        yps[:, :w], lhsT=h1T_p1[:, k * BATCH:(k + 1) * BATCH],
                        rhs=wy[:, k * VS + n0:k * VS + n1],
                        start=(k == 0), stop=(not has_by and k == KH - 1))
                if has_by:
                    nc.tensor.matmul(yps[:, :w], lhsT=ones[:, :BATCH],
                                     rhs=byr[:, n0:n1], start=False, stop=True)
                nc.vector.tensor_copy(ysb[:, n0:n1], yps[:, :w])
            nc.sync.dma_start(y_out.ap(), ysb[:])

    nc.compile()
    return nc


def _get_built(flags):
    global _BUILT
    if _BUILT is None:
        _BUILT = _build(NSTEPS, *flags)
    return _BUILT


# ---------------------------------------------------------------- runner

def _install_trace_hook():
    import types
    try:
        from trn_agent_boot.trn_boot import _ntff_profile_via_ctypes
        import antenv  # noqa: F401
        hook = _ntff_profile_via_ctypes('/opt/axon/libaxon_pjrt.so')
        mod = types.ModuleType('antenv.axon_hooks')
        mod.get_axon_ntff_profile_hook = lambda: hook
        sys.modules['antenv.axon_hooks'] = mod
        return True
    except Exception:
        return False


def kernel(texts, emb, Wf0, bf0, Wi0, bi0, Wo0, bo0, Wc0, bc0,
           Wf1, bf1, Wi1, bi1, Wo1, bo1, Wc1, bc1, Wy, by):
    global LAST_EXEC_NS
    from concourse import bass_utils

    texts = np.asarray(texts)
    emb = np.asarray(emb, np.float32)
    shards, flags = _prep_shards(Wf0, Wi0, Wo0, Wc0, bf0, bi0, bo0, bc0,
                                 Wf1, Wi1, Wo1, Wc1, bf1, bi1, bo1, bc1,
                                 Wy, by)

    nsteps = NSTEPS
    ntok = nsteps * BATCH
    ntiles = (ntok + 127) // 128
    offs = np.zeros(ntiles * 128, np.int32)
    offs[:ntok] = texts.T[:nsteps, :].reshape(-1).astype(np.int32)
    emb_bf = emb.astype(BF16)
    ones = np.ones((1, 128), BF16)
    ident = np.eye(128, dtype=BF16)

    in_maps = []
    for j in range(NCORES):
        sh = shards[j]
        in_maps.append({
            "offs": offs, "emb_bf": emb_bf,
            "WH0": sh["WH0"], "WX0": sh["WX0"],
            "WH1": sh["WH1"], "WX1": sh["WX1"],
            "B0": sh["B0"], "B1": sh["B1"],
            "WY": sh["WY"], "BY": sh["BY"],
            "ONES": ones, "IDENT": ident,
        })

    nc = _get_built(flags)
    kwargs = {}
    if TRACE:
        if _install_trace_hook():
            import tempfile
            kwargs = dict(trace=True,
                          tmpdir=tempfile.mkdtemp(prefix="lstm_trace_"))
    res = bass_utils.run_bass_kernel_spmd(
        nc, in_maps, core_ids=list(range(NCORES)), **kwargs)
    LAST_EXEC_NS = res.exec_time_ns
    y = np.concatenate([res.results[j]["y"] for j in range(NCORES)], axis=1)
    return y.astype(np.float32)


if __name__ == "__main__":
    pass


# revision 26
# speedup vs baseline: 6.8806x; 6.8765x over previous
"""nn_CustomLSTMModel kernel: 2-layer LSTM (H=1024) over (B=64, T=512) tokens.

Strategy: tensor-parallel over the gate/hidden dimension across 8 NeuronCores.
Each core j owns hidden slice [j*128, (j+1)*128) of both layers: it holds the
weight rows for its slice (transposed, bf16), computes its slice of the gates
for the FULL batch (64), updates its local cell-state slice, and exchanges its
h-slice with the other cores via one AllGather per superstep (both layers'
chunks ride one collective).  The layers run as a lag-2 wavefront: superstep s
computes h0(s) and h1(s-2), so layer 1's input-side matmuls (on h0(s-2)) and
layer 0's input-side matmuls (on the pre-gathered token embeddings) are not
gated on the in-flight AllGather — they fill the PE during the collective and
keep the HAM clock warm.  Embeddings are gathered on device (indirect DMA)
and PE-transposed into an SBUF ring ahead of consumption.  The final vocab
projection is sharded over cores (4000 rows each); the host concatenates.

Weights stream through the PE as the moving operand in bf16 (1 cycle/row);
h stays transposed [H, B] in SBUF so it can be the stationary operand.
Cell state c stays fp32 and core-local (never communicated).
"""

import os
import sys
import numpy as np

for _p in ("/opt/trn_rl_repo", "/opt/pypackages"):
    if _p not in sys.path:
        sys.path.insert(0, _p)

import ml_dtypes

BF16 = ml_dtypes.bfloat16

VOCAB, EMBED, HIDDEN, BATCH, SEQ = 32000, 512, 1024, 64, 512
NCORES = 8
HS = HIDDEN // NCORES           # 128: hidden slice per core
GS = 4 * HS                     # 512: gate rows per core (f,i,o,c x 128)
VS = VOCAB // NCORES            # 4000: vocab slice per core
KH = HIDDEN // 128              # 8 contraction chunks over H
KE = EMBED // 128               # 4 contraction chunks over E
NSTEPS = int(os.environ.get("LSTM_STEPS", str(SEQ)))
LOOKAHEAD = 6                   # prologue tiles (2 steps each) emitted ahead

TRACE = False          # set by test.py; when True, run traced, set LAST_EXEC_NS
LAST_EXEC_NS = None

_BUILT = None          # cached {key: nc}


# ---------------------------------------------------------------- host prep

def _prep_shards(Wf0, Wi0, Wo0, Wc0, bf0, bi0, bo0, bc0,
                 Wf1, Wi1, Wo1, Wc1, bf1, bi1, bo1, bc1, Wy, by):
    """Build per-core weight shards in the on-chip layouts."""
    gates0 = [np.asarray(w, np.float32) for w in (Wf0, Wi0, Wo0, Wc0)]
    gates1 = [np.asarray(w, np.float32) for w in (Wf1, Wi1, Wo1, Wc1)]
    b0 = [np.asarray(b, np.float32) for b in (bf0, bi0, bo0, bc0)]
    b1 = [np.asarray(b, np.float32) for b in (bf1, bi1, bo1, bc1)]
    Wy = np.asarray(Wy, np.float32)
    by = np.asarray(by, np.float32)

    def chunked(W):  # [K, N] -> [128, (K//128)*N], chunk k at cols [k*N,(k+1)*N)
        K, N = W.shape
        return np.ascontiguousarray(
            W.reshape(K // 128, 128, N).transpose(1, 0, 2).reshape(128, -1)
        )

    shards = []
    for j in range(NCORES):
        r = slice(j * HS, (j + 1) * HS)
        Wh0 = np.concatenate([g[r, :HIDDEN].T for g in gates0], axis=1)  # (1024,512)
        Wx0 = np.concatenate([g[r, HIDDEN:].T for g in gates0], axis=1)  # (512,512)
        Wh1 = np.concatenate([g[r, :HIDDEN].T for g in gates1], axis=1)  # (1024,512)
        Wx1 = np.concatenate([g[r, HIDDEN:].T for g in gates1], axis=1)  # (1024,512)
        b0row = np.concatenate([b[r] for b in b0])[None, :]
        b1row = np.concatenate([b[r] for b in b1])[None, :]

        v = slice(j * VS, (j + 1) * VS)
        WYj = np.ascontiguousarray(
            Wy[v, :].T.reshape(KH, 128, VS).transpose(1, 0, 2).reshape(128, -1)
        )
        byrow = by[v][None, :]

        shards.append(dict(
            WH0=chunked(Wh0).astype(BF16), WX0=chunked(Wx0).astype(BF16),
            WH1=chunked(Wh1).astype(BF16), WX1=chunked(Wx1).astype(BF16),
            B0=b0row.astype(BF16), B1=b1row.astype(BF16),
            WY=WYj.astype(BF16), BY=byrow.astype(BF16),
        ))
    has_b0 = any(np.any(b != 0) for b in b0)
    has_b1 = any(np.any(b != 0) for b in b1)
    has_by = bool(np.any(by != 0))
    return shards, (has_b0, has_b1, has_by)


# ---------------------------------------------------------------- builder

def _build(nsteps, has_b0, has_b1, has_by):
    import concourse.bacc as bacc
    import concourse.bass as bass
    import concourse.mybir as mybir
    import concourse.tile as tile

    dt = mybir.dt
    AF = mybir.ActivationFunctionType
    NTOK = nsteps * BATCH
    NTILES = (NTOK + 127) // 128          # 128-token prologue tiles (2 steps)

    nc = bacc.Bacc("TRN2", target_bir_lowering=False, debug=False,
                   enable_asserts=True, num_devices=NCORES)

    offs = nc.dram_tensor("offs", [NTILES * 128], dt.int32, kind="ExternalInput")
    emb_bf = nc.dram_tensor("emb_bf", [VOCAB, EMBED], dt.bfloat16,
                            kind="ExternalInput")
    WH0 = nc.dram_tensor("WH0", [128, KH * GS], dt.bfloat16, kind="ExternalInput")
    WX0 = nc.dram_tensor("WX0", [128, KE * GS], dt.bfloat16, kind="ExternalInput")
    WH1 = nc.dram_tensor("WH1", [128, KH * GS], dt.bfloat16, kind="ExternalInput")
    WX1 = nc.dram_tensor("WX1", [128, KH * GS], dt.bfloat16, kind="ExternalInput")
    B0 = nc.dram_tensor("B0", [1, GS], dt.bfloat16, kind="ExternalInput")
    B1 = nc.dram_tensor("B1", [1, GS], dt.bfloat16, kind="ExternalInput")
    WY = nc.dram_tensor("WY", [128, KH * VS], dt.bfloat16, kind="ExternalInput")
    BY = nc.dram_tensor("BY", [1, VS], dt.bfloat16, kind="ExternalInput")
    ONES = nc.dram_tensor("ONES", [1, 128], dt.bfloat16, kind="ExternalInput")
    IDENT = nc.dram_tensor("IDENT", [128, 128], dt.bfloat16, kind="ExternalInput")
    y_out = nc.dram_tensor("y", [BATCH, VS], dt.float32, kind="ExternalOutput")

    with tile.TileContext(nc) as tc:
        with (
            tc.tile_pool(name="wpool", bufs=1) as wpool,
            tc.tile_pool(name="state", bufs=1) as state,
            tc.tile_pool(name="work", bufs=3) as work,
            tc.tile_pool(name="xring", bufs=LOOKAHEAD + 3) as xring,
            tc.tile_pool(name="hbuf", bufs=4) as hbuf,
            tc.tile_pool(name="psum", bufs=2, space="PSUM") as psum,
            tc.tile_pool(name="tpsum", bufs=3, space="PSUM") as tpsum,
            tc.tile_pool(name="jpsum", bufs=1, space="PSUM") as jpsum,
            tc.tile_pool(name="agd", bufs=2, space="DRAM") as agd,
        ):
            # ---- persistent weights/constants ----
            def load(tname, src, shape, dtype=dt.bfloat16):
                t = wpool.tile(shape, dtype, tag=tname)
                nc.sync.dma_start(t[:], src.ap())
                return t

            wh0 = load("wh0", WH0, [128, KH * GS])
            wx0 = load("wx0", WX0, [128, KE * GS])
            wh1 = load("wh1", WH1, [128, KH * GS])
            wx1 = load("wx1", WX1, [128, KH * GS])
            b0r = load("b0r", B0, [1, GS])
            b1r = load("b1r", B1, [1, GS])
            wy = load("wy", WY, [128, KH * VS])
            byr = load("byr", BY, [1, VS])
            ones = load("ones", ONES, [1, 128])
            ident = load("ident", IDENT, [128, 128])
            offs_t = wpool.tile([128, NTILES], dt.int32, tag="offs")
            nc.sync.dma_start(
                offs_t[:], offs.ap().rearrange("(t p) -> p t", p=128))

            # ---- persistent state ----
            c0 = state.tile([BATCH, HS], dt.float32, tag="c0")
            c1 = state.tile([BATCH, HS], dt.float32, tag="c1")
            zro = state.tile([128, BATCH], dt.bfloat16, tag="zro")
            nc.vector.memset(c0[:], 0.0)
            nc.vector.memset(c1[:], 0.0)
            nc.vector.memset(zro[:], 0.0)

            h0T_init = hbuf.tile([128, KH * BATCH], dt.bfloat16, tag="h0T")
            h1T_init = hbuf.tile([128, KH * BATCH], dt.bfloat16, tag="h1T")
            nc.vector.memset(h0T_init[:], 0.0)
            nc.vector.memset(h1T_init[:], 0.0)

            x_tiles = [None] * NTILES

            # ---- prologue tile: gather 128 tokens, transpose to xsT ring ----
            def emit_x_tile(tau):
                ntok = min(128, NTOK - tau * 128)
                xg = work.tile([128, EMBED], dt.bfloat16, tag="xg")
                nc.gpsimd.indirect_dma_start(
                    out=xg[:ntok, :], out_offset=None,
                    in_=emb_bf.ap(),
                    in_offset=bass.IndirectOffsetOnAxis(
                        ap=offs_t[:ntok, tau:tau + 1], axis=0),
                )
                xT = xring.tile([128, KE * 128], dt.bfloat16, tag="xT")
                for ch in range(KE):
                    tp = tpsum.tile([128, 128], dt.bfloat16, tag="tp")
                    nc.tensor.transpose(
                        out=tp[:, :ntok], in_=xg[:ntok, ch * 128:(ch + 1) * 128],
                        identity=ident[:ntok, :ntok])
                    nc.vector.tensor_copy(
                        xT[:, ch * 128:ch * 128 + ntok], tp[:, :ntok])
                x_tiles[tau] = xT

            for tau in range(min(LOOKAHEAD, NTILES)):
                emit_x_tile(tau)

            # ---- EW: gates PSUM [64, GS] -> h chunk bf16 + c update ----
            def lstm_ew(ps, c, tag):
                sio = work.tile([BATCH, 3 * HS], dt.float32, tag=f"sio{tag}")
                th = work.tile([BATCH, HS], dt.float32, tag=f"th{tag}")
                nc.scalar.activation(sio[:], ps[:, 0:3 * HS], AF.Sigmoid)
                nc.scalar.activation(th[:], ps[:, 3 * HS:4 * HS], AF.Tanh)
                t1 = work.tile([BATCH, HS], dt.float32, tag=f"t1{tag}")
                nc.vector.tensor_mul(t1[:], sio[:, 0:HS], c[:])          # f*c
                t2 = work.tile([BATCH, HS], dt.float32, tag=f"t2{tag}")
                nc.vector.tensor_mul(t2[:], sio[:, HS:2 * HS], th[:])    # i*ch
                nc.vector.tensor_add(c[:], t1[:], t2[:])
                tc_ = work.tile([BATCH, HS], dt.float32, tag=f"tc{tag}")
                nc.scalar.activation(tc_[:], c[:], AF.Tanh)
                hj = work.tile([BATCH, HS], dt.bfloat16, tag=f"hj{tag}")
                nc.vector.tensor_mul(hj[:], sio[:, 2 * HS:3 * HS], tc_[:])
                tp = tpsum.tile([128, BATCH], dt.bfloat16, tag="tp")
                nc.tensor.transpose(out=tp[:], in_=hj[:],
                                    identity=ident[:BATCH, :BATCH])
                hT = work.tile([128, BATCH], dt.bfloat16, tag=f"hT{tag}")
                nc.vector.tensor_copy(hT[:], tp[:])
                return hT

            # ---- recurrence: superstep s computes h0(s) and h1(s-2) ----
            h0T_p1 = h0T_init   # h0T from AG(s-1)
            h0T_p2 = h0T_init   # h0T from AG(s-2)
            h1T_p1 = h1T_init   # h1T from AG(s-1) = h1(s-3)
            nsup = nsteps + 2
            for s in range(1, nsup + 1):
                do_l0 = s <= nsteps
                do_l1 = s >= 3

                # ---------- ungated work (runs during AG(s-1) flight) ----------
                if do_l1:
                    # layer 1, step s-2: x-part = hs0(s-2) @ Wx1
                    ps1 = psum.tile([BATCH, GS], dt.float32, tag="g1ps")
                    for k in range(KH):
                        nc.tensor.matmul(
                            ps1[:], lhsT=h0T_p2[:, k * BATCH:(k + 1) * BATCH],
                            rhs=wx1[:, k * GS:(k + 1) * GS],
                            start=(k == 0), stop=False)
                    if has_b1:
                        nc.tensor.matmul(ps1[:], lhsT=ones[:, :BATCH],
                                         rhs=b1r[:], start=False, stop=False)
                if do_l0:
                    # layer 0, step s: x-part = xs(s) @ Wx0 (+ b0)
                    ps0 = psum.tile([BATCH, GS], dt.float32, tag="g0ps")
                    tau, off = (s - 1) // 2, ((s - 1) % 2) * BATCH
                    xT = x_tiles[tau]
                    for ch in range(KE):
                        nc.tensor.matmul(
                            ps0[:], lhsT=xT[:, ch * 128 + off:ch * 128 + off + BATCH],
                            rhs=wx0[:, ch * GS:(ch + 1) * GS],
                            start=(ch == 0), stop=False)
                    if has_b0:
                        nc.tensor.matmul(ps0[:], lhsT=ones[:, :BATCH],
                                         rhs=b0r[:], start=False, stop=False)

                # ---------- gated on readbacks of AG0/AG1(s-1) ----------
                # L1 first: its AG1 triggers as soon as EW1 finishes; AG1's
                # result is only needed next superstep, so its latency hides.
                if do_l1:
                    # layer 1 h-part: h1(s-3) @ Wh1  (h1T_p1 from AG1(s-1))
                    for k in range(KH):
                        nc.tensor.matmul(
                            ps1[:], lhsT=h1T_p1[:, k * BATCH:(k + 1) * BATCH],
                            rhs=wh1[:, k * GS:(k + 1) * GS],
                            start=False, stop=(k == KH - 1))
                    hT1 = lstm_ew(ps1, c1, "1")
                    agin1 = agd.tile([128, BATCH], dt.bfloat16, tag="agin1")
                    agout1 = agd.tile([NCORES * 128, BATCH], dt.bfloat16,
                                      tag="agout1")
                    nc.sync.dma_start(agin1[:], hT1[:])
                    nc.gpsimd.collective_compute(
                        "AllGather", mybir.AluOpType.bypass,
                        replica_groups=[list(range(NCORES))],
                        ins=[agin1.opt()], outs=[agout1.opt()])
                    h1T_new = hbuf.tile([128, KH * BATCH], dt.bfloat16,
                                        tag="h1T")
                    nc.sync.dma_start(
                        h1T_new[:].rearrange("p (r b) -> p r b", r=NCORES),
                        agout1[:].rearrange("(r p) b -> p r b",
                                            r=NCORES, p=128))
                    h1T_p1 = h1T_new

                if do_l0:
                    for k in range(KH):
                        nc.tensor.matmul(
                            ps0[:], lhsT=h0T_p1[:, k * BATCH:(k + 1) * BATCH],
                            rhs=wh0[:, k * GS:(k + 1) * GS],
                            start=False, stop=(k == KH - 1))
                    hT0 = lstm_ew(ps0, c0, "0")
                    agin0 = agd.tile([128, BATCH], dt.bfloat16, tag="agin0")
                    agout0 = agd.tile([NCORES * 128, BATCH], dt.bfloat16,
                                      tag="agout0")
                    nc.sync.dma_start(agin0[:], hT0[:])
                    nc.gpsimd.collective_compute(
                        "AllGather", mybir.AluOpType.bypass,
                        replica_groups=[list(range(NCORES))],
                        ins=[agin0.opt()], outs=[agout0.opt()])
                    h0T_new = hbuf.tile([128, KH * BATCH], dt.bfloat16,
                                        tag="h0T")
                    nc.sync.dma_start(
                        h0T_new[:].rearrange("p (r b) -> p r b", r=NCORES),
                        agout0[:].rearrange("(r p) b -> p r b",
                                            r=NCORES, p=128))
                    h0T_p2, h0T_p1 = h0T_p1, h0T_new
                else:
                    h0T_p2 = h0T_p1

                # gather lands behind the collective on the gpsimd queue, so
                # it runs during the AG flight instead of delaying the trigger
                tau_need = LOOKAHEAD + (s - 1) // 2
                if tau_need < NTILES and x_tiles[tau_need] is None:
                    emit_x_tile(tau_need)

                # HAM pacer: junk matmul bursts paced by DMA latency fragment
                # the PE idle during the AG flights below the ~3.4us MID
                # window, so the PE clock stays at 2.4 GHz.  The trigger DMAs
                # ride the (idle) vector queue; the bursts sit at the end of
                # the PE queue so they never delay real matmuls.
                if do_l0 and do_l1:
                    rungs = []
                    j1 = work.tile([128, BATCH], dt.bfloat16, tag="j1")
                    nc.scalar.dma_start(j1[:], agin1[:])
                    rungs.append(j1)
                    j2 = work.tile([128, BATCH], dt.bfloat16, tag="j2")
                    nc.scalar.dma_start(j2[:], agin0[:])
                    rungs.append(j2)
                    j3 = work.tile([128, BATCH], dt.bfloat16, tag="j3")
                    nc.scalar.dma_start(j3[:], j2[:])
                    rungs.append(j3)
                    j4 = work.tile([128, BATCH], dt.bfloat16, tag="j4")
                    nc.scalar.dma_start(j4[:], j3[:])
                    rungs.append(j4)
                    jp = jpsum.tile([BATCH, GS], dt.float32, tag="jp")
                    for i, src in enumerate(rungs):
                        for r in range(3):
                            nc.tensor.matmul(
                                jp[:], lhsT=src[:, :BATCH],
                                rhs=wh0[:, r * GS:(r + 1) * GS],
                                start=(r == 0), stop=(r == 2),
                                skip_group_check=True)

            # ---- final projection: y_j = h1(T) @ Wy_j.T + by_j ----
            NCH = (VS + 499) // 500
            ysb = state.tile([BATCH, VS], dt.float32, tag="ysb")
            for nchunk in range(NCH):
                n0 = nchunk * 500
                n1 = min(n0 + 500, VS)
                w = n1 - n0
                yps = psum.tile([BATCH, 500], dt.float32, tag="g0ps")
                for k in range(KH):
                    nc.tensor.matmul(
                        yps[:, :w], lhsT=h1T_p1[:, k * BATCH:(k + 1) * BATCH],
                        rhs=wy[:, k * VS + n0:k * VS + n1],
                        start=(k == 0), stop=(not has_by and k == KH - 1))
                if has_by:
                    nc.tensor.matmul(yps[:, :w], lhsT=ones[:, :BATCH],
                                     rhs=byr[:, n0:n1], start=False, stop=True)
                nc.vector.tensor_copy(ysb[:, n0:n1], yps[:, :w])
            nc.sync.dma_start(y_out.ap(), ysb[:])

    nc.compile()
    return nc


def _get_built(flags):
    global _BUILT
    if _BUILT is None:
        _BUILT = _build(NSTEPS, *flags)
    return _BUILT


# ---------------------------------------------------------------- runner

def _install_trace_hook():
    import types
    try:
        from trn_agent_boot.trn_boot import _ntff_profile_via_ctypes
        import antenv  # noqa: F401
        hook = _ntff_profile_via_ctypes('/opt/axon/libaxon_pjrt.so')
        mod = types.ModuleType('antenv.axon_hooks')
        mod.get_axon_ntff_profile_hook = lambda: hook
        sys.modules['antenv.axon_hooks'] = mod
        return True
    except Exception:
        return False


def kernel(texts, emb, Wf0, bf0, Wi0, bi0, Wo0, bo0, Wc0, bc0,
           Wf1, bf1, Wi1, bi1, Wo1, bo1, Wc1, bc1, Wy, by):
    global LAST_EXEC_NS
    from concourse import bass_utils

    texts = np.asarray(texts)
    emb = np.asarray(emb, np.float32)
    shards, flags = _prep_shards(Wf0, Wi0, Wo0, Wc0, bf0, bi0, bo0, bc0,
                                 Wf1, Wi1, Wo1, Wc1, bf1, bi1, bo1, bc1,
                                 Wy, by)

    nsteps = NSTEPS
    ntok = nsteps * BATCH
    ntiles = (ntok + 127) // 128
    offs = np.zeros(ntiles * 128, np.int32)
    offs[:ntok] = texts.T[:nsteps, :].reshape(-1).astype(np.int32)
    emb_bf = emb.astype(BF16)
    ones = np.ones((1, 128), BF16)
    ident = np.eye(128, dtype=BF16)

    in_maps = []
    for j in range(NCORES):
        sh = shards[j]
        in_maps.append({
            "offs": offs, "emb_bf": emb_bf,
            "WH0": sh["WH0"], "WX0": sh["WX0"],
            "WH1": sh["WH1"], "WX1": sh["WX1"],
            "B0": sh["B0"], "B1": sh["B1"],
            "WY": sh["WY"], "BY": sh["BY"],
            "ONES": ones, "IDENT": ident,
        })

    nc = _get_built(flags)
    kwargs = {}
    if TRACE:
        if _install_trace_hook():
            import tempfile
            kwargs = dict(trace=True,
                          tmpdir=tempfile.mkdtemp(prefix="lstm_trace_"))
    res = bass_utils.run_bass_kernel_spmd(
        nc, in_maps, core_ids=list(range(NCORES)), **kwargs)
    LAST_EXEC_NS = res.exec_time_ns
    y = np.concatenate([res.results[j]["y"] for j in range(NCORES)], axis=1)
    return y.astype(np.float32)


if __name__ == "__main__":
    pass
